# revision 1
# baseline (speedup 1.0000x reference)
"""Trainium2 Bass kernel: DGCNN Zernike-monomial interwiner (nn_DGCNN_8839042695322).

Computes, per point p=(x,y,z):
  out[.., 16, 4] = concat_l( einsum(zernike_monoms(p)[l], Wl) ) for l=0..3
Every output channel is a degree<=3 polynomial in (x,y,z); all weights are
folded host-side into per-channel scalar immediates (the compiled program is
cached per weight set).

Memory-bound. Precision strategy (correctness gate is rel_err < 2e-2):
  - l0/l1/l3 channels (94% of output energy): fp16, rel err ~2e-4
  - l2 channels (0.25% of output energy, |v| < 2): fp8 e4m3, adds ~1.3e-3
  - net measured rel err ~1.5e-3, 13x under the gate
This cuts HBM write traffic 2.4x vs f32 (16.8MB -> 14.2MB per core vs 33.5).
The host upcasts/decodes to f32 during unsharding.

The device output is channel-major [rows x T points] so every compute op is
fully contiguous; whole per-degree unit blocks are written with single wide
ops. Output DMA is chunked by row-group, issued in expected completion order
(the Sync queue is in-order; a mis-ordered chunk head-of-line blocks later
ones). Chunk count is kept moderate: every hardware-dynamic dma_start costs
descriptor-fetch bandwidth on DMA engine 79, which otherwise straggles.

Engine notes (TRN2): DVE tensor_tensor fp16 runs 2x (N/2+143 cyc),
tensor_scalar 4x (N/4+143), scalar_tensor_tensor only 1x (N+143); Scalar
engine ops cost (N+352)/1.2GHz regardless of dtype -- so it takes the wide
l2 fp8 blocks (free dtype conversion) and one l3 block. Each iteration
builds the l2 bases first so the Scalar stream starts as early as possible,
then drains l0/l1 while the Scalar engine works, then the l3 chain.

Sharding: pure data parallel over the batch axis across 8 NeuronCores.
"""

import numpy as np

import concourse.bacc as bacc
import concourse.tile as tile
from concourse import mybir
from concourse.bass_utils import run_bass_kernel_spmd

# Problem geometry (hardcoded per spec: x [32, 32768, 3] f32, 8 cores).
B, N, M_CORES = 32, 32768, 8
PTS_PER_CORE = B * N // M_CORES  # 131072
P = 128                          # SBUF partitions
COLS = PTS_PER_CORE // P         # 1024 points per partition
ITER_LENS = [512, 512]
assert sum(ITER_LENS) == COLS

# GPSIMD scalar_tensor_tensor fails codegen ("engine check failed (Pool)");
# keep the aux planes on DVE.
GPSIMD_AUX = False

# Real spherical-harmonic constants (match reference).
C0 = 0.28209479177387814
C1 = 0.4886025119029199
C2_XY = 1.0925484305920792
C2_0 = 0.31539156525252005
C2_2 = 0.5462742152960396
C3_3 = 0.5900435899266435
C3_2 = 2.890611442640554
C3_1 = 0.4570457994644658
C3_0 = 0.3731763325901154
C3_P2 = 1.445305721320277

# fp16 tensor rows (44): 0..3 l0 (u), 4..15 l1 (m,u) m-major,
#   16..43 l3 (u,m) u-major.  fp8 tensor rows (20): l2 (u,m) u-major.
# Final channel ch=(m*4+u): m=0 -> f16 row u; m in 1..3 -> f16 row ch;
#   m in 4..8 -> fp8 row 5u+(m-4); m in 9..15 -> f16 row 16+7u+(m-9).
CH16 = [c for c in range(64) if (c // 4) < 4 or (c // 4) >= 9]
CH8 = [c for c in range(64) if 4 <= (c // 4) < 9]
IDX16 = np.array(
    [(c % 4) if c // 4 == 0 else
     (c if c // 4 < 4 else 16 + 7 * (c % 4) + (c // 4 - 9))
     for c in CH16], dtype=np.int64)
IDX8 = np.array([5 * (c % 4) + (c // 4 - 4) for c in CH8], dtype=np.int64)

_cache: dict = {}


def _host_constants(W0, b0, W1, W2, W3):
    """Fold interwiner weights into per-channel scalars (f64 host math)."""
    A0 = (C0 * W0[0].astype(np.float64) + b0.astype(np.float64)).astype(np.float32)
    B0 = (C0 * W0[1].astype(np.float64)).astype(np.float32)
    AA1 = (C1 * W1[0].astype(np.float64)).astype(np.float32)
    BB1 = (C1 * W1[1].astype(np.float64)).astype(np.float32)
    w2u = W2[0].astype(np.float64).astype(np.float32)  # [4]
    w3u = W3[0].astype(np.float64).astype(np.float32)  # [4]
    return dict(A0=A0, B0=B0, AA1=AA1, BB1=BB1, w2u=w2u, w3u=w3u)


def _build_program(consts, iter_lens=None):
    iter_lens = list(iter_lens or ITER_LENS)
    f16 = mybir.dt.float16
    f8 = mybir.dt.float8e4
    F = mybir.ActivationFunctionType
    ALU = mybir.AluOpType
    A0, B0 = consts["A0"], consts["B0"]
    AA1, BB1 = consts["AA1"], consts["BB1"]
    w2u, w3u = consts["w2u"], consts["w3u"]

    nc = bacc.Bacc(
        "TRN2", target_bir_lowering=False, debug=False, num_devices=M_CORES
    )
    xin = nc.dram_tensor("xin", [P, 3 * COLS], f16, kind="ExternalInput").ap()
    y16 = nc.dram_tensor("y16", [P, 44 * COLS], f16, kind="ExternalOutput").ap()
    y8 = nc.dram_tensor("y8", [P, 20 * COLS], f8, kind="ExternalOutput").ap()

    nb = len(iter_lens)
    with tile.TileContext(nc) as tc:
        with (
            tc.tile_pool(name="xp", bufs=nb) as xp,
            tc.tile_pool(name="zp", bufs=nb) as zp,
            tc.tile_pool(name="wk", bufs=2) as wk,
            tc.tile_pool(name="op", bufs=2) as op_,
        ):
            # Phase A: input loads + z-squares on ACT (hoisted so a later
            # iteration's n2 chain never waits behind ACT's wide copies).
            xts, z2s = [], []
            ts = 0
            for it, T in enumerate(iter_lens):
                xt = xp.tile([P, 3 * T], f16, name=f"xt{it}")
                nc.sync.dma_start(out=xt, in_=xin[:, 3 * ts : 3 * (ts + T)])
                z2 = zp.tile([P, T], f16, name=f"z2_{it}")
                nc.scalar.activation(z2, xt[:, 2 * T : 3 * T], F.Square)
                xts.append(xt)
                z2s.append(z2)
                ts += T

            # Phase B: per-iteration compute + chunked output DMA.
            ts = 0
            for it, T in enumerate(iter_lens):
                xt, z2 = xts[it], z2s[it]
                px, py, pz = xt[:, 0:T], xt[:, T : 2 * T], xt[:, 2 * T : 3 * T]

                def pl(tag, k=1):
                    return wk.tile([P, k * T], f16, name=tag)

                x2, y2 = pl("x2"), pl("y2")
                n2a, n2 = pl("n2a"), pl("n2")
                t2a, x2my2 = pl("t2a"), pl("x2my2")
                a3, b3, cn2, c3s, d3 = (
                    pl("a3"), pl("b3"), pl("cn2"), pl("c3s"), pl("d3")
                )
                sp = pl("sp", 4)
                cxy = pl("cxy", 2)
                b2 = pl("b2", 5)
                bl3 = pl("bl3", 7)
                ot = op_.tile([P, 44 * T], f16, name="ot")
                o8 = op_.tile([P, 20 * T], f8, name="o8")

                def orow(r, k=1):
                    return ot[:, r * T : (r + k) * T]

                def row(buf, r, k=1):
                    return buf[:, r * T : (r + k) * T]

                def odma16(r0, r1):
                    nc.sync.dma_start(
                        out=y16[:, 44 * ts + r0 * T : 44 * ts + r1 * T],
                        in_=orow(r0, r1 - r0),
                    )

                def odma8(r0, r1):
                    nc.sync.dma_start(
                        out=y8[:, 20 * ts + r0 * T : 20 * ts + r1 * T],
                        in_=o8[:, r0 * T : r1 * T],
                    )

                STT = nc.vector.scalar_tensor_tensor
                TS = nc.vector.tensor_scalar
                TT_MUL = nc.vector.tensor_mul

                # --- n2 chain, then b2 first: unblocks ACT's l2w stream
                # ~2.3us earlier than computing the l0/l1 path first ---
                TT_MUL(x2, px, px)
                TT_MUL(y2, py, py)
                nc.vector.tensor_add(n2a, x2, y2)
                nc.vector.tensor_add(n2, n2a, z2)
                for u in range(4):
                    TS(orow(u), n2, float(B0[u]), float(A0[u]),
                       op0=ALU.mult, op1=ALU.add)
                odma16(0, 4)
                STT(t2a, z2, 3.0, n2, op0=ALU.mult, op1=ALU.subtract)
                nc.vector.tensor_sub(x2my2, x2, y2)
                TS(row(b2, 2), t2a, float(C2_0), None, op0=ALU.mult)
                TS(row(b2, 4), x2my2, float(C2_2), None, op0=ALU.mult)
                TS(cxy, xt[:, 0 : 2 * T], float(C2_XY), None, op0=ALU.mult)
                TT_MUL(row(b2, 0), row(cxy, 0), py)   # C*px*py
                TT_MUL(row(b2, 1), row(cxy, 1), pz)   # C*py*pz
                TT_MUL(row(b2, 3), row(cxy, 0), pz)   # C*px*pz

                # --- l2 wide unit blocks on ACT -> fp8 ---
                for u in range(4):
                    nc.scalar.activation(
                        o8[:, 5 * u * T : 5 * (u + 1) * T], b2, F.Copy,
                        scale=float(w2u[u]),
                    )

                # --- l1 (rows 4..15), drained in two pieces so the DMA has
                # bytes during ACT's l2w phase ---
                for u in range(4):
                    TS(row(sp, u), n2, float(BB1[u]), float(AA1[u]),
                       op0=ALU.mult, op1=ALU.add)
                sp3 = sp.rearrange("p (a b) -> p a b", a=4)
                for mi, pm in enumerate((py, pz, px)):
                    pmb = pm.unsqueeze(1).broadcast_to([P, 4, T])
                    TT_MUL(
                        orow(4 + 4 * mi, 4).rearrange("p (a b) -> p a b", a=4),
                        sp3, pmb,
                    )
                    if mi == 1:
                        odma16(4, 12)
                odma16(12, 16)
                odma8(0, 20)

                # --- l3 bases ---
                STT(a3, x2, 3.0, y2, op0=ALU.mult, op1=ALU.subtract)
                STT(b3, y2, -3.0, x2, op0=ALU.mult, op1=ALU.add)
                STT(d3, n2, -0.6, z2, op0=ALU.mult, op1=ALU.add)
                TS(cn2, n2, float(C3_1), None, op0=ALU.mult)
                STT(c3s, z2, 5.0 * C3_1, cn2, op0=ALU.mult, op1=ALU.subtract)
                STT(row(bl3, 0), py, C3_3, a3, op0=ALU.mult, op1=ALU.mult)
                STT(row(bl3, 1), pz, C3_2 / C2_XY, row(b2, 0),
                    op0=ALU.mult, op1=ALU.mult)
                TT_MUL(row(bl3, 2), py, c3s)
                STT(row(bl3, 3), pz, 5.0 * C3_0, d3,
                    op0=ALU.mult, op1=ALU.mult)
                TT_MUL(row(bl3, 4), px, c3s)
                STT(row(bl3, 5), pz, C3_P2 / C2_2, row(b2, 4),
                    op0=ALU.mult, op1=ALU.mult)
                STT(row(bl3, 6), px, C3_3, b3, op0=ALU.mult, op1=ALU.mult)

                # --- l3 wide unit blocks (f16 rows 16..43), split odmas so
                # the DVE part (units 1..3) drains without waiting for ACT ---
                for u in (1, 2, 3):
                    TS(orow(16 + 7 * u, 7), bl3, float(w3u[u]), None,
                       op0=ALU.mult)
                odma16(23, 44)
                nc.scalar.activation(
                    orow(16, 7), bl3, F.Copy, scale=float(w3u[0])
                )
                odma16(16, 23)
                ts += T

    nc.compile()
    return nc


def _get_program(consts, iter_lens=None):
    key = tuple(
        consts[k].tobytes() for k in ("A0", "B0", "AA1", "BB1", "w2u", "w3u")
    ) + (tuple(iter_lens or ITER_LENS), GPSIMD_AUX, "fp8l2")
    if _cache.get(key) is None:
        _cache[key] = _build_program(consts, iter_lens)
    return _cache[key]


def _prep_inputs(x, iter_lens):
    """[B,N,3] f32 -> per-core [P, 3*COLS] fp16, tile-major blocks [3,T]."""
    xs = np.asarray(x, dtype=np.float32).reshape(M_CORES, P, COLS, 3)
    xs = np.ascontiguousarray(xs.transpose(0, 1, 3, 2)).astype(np.float16)
    parts = []
    ts = 0
    for T in iter_lens:
        parts.append(xs[:, :, :, ts : ts + T].reshape(M_CORES, P, 3 * T))
        ts += T
    return np.ascontiguousarray(np.concatenate(parts, axis=2))


def _reconstruct(results, iter_lens):
    """Per-core channel-major fp16+fp8 -> full [B,N,16,4] f32."""
    out = np.empty((M_CORES, P, COLS, 64), dtype=np.float32)
    for c in range(M_CORES):
        a16 = results[c]["y16"]
        a8 = results[c]["y8"]
        ts = 0
        for T in iter_lens:
            blk = a16[:, 44 * ts : 44 * (ts + T)].reshape(P, 44, T)
            out[c, :, ts : ts + T, CH16] = (
                blk[:, IDX16, :].transpose(1, 0, 2))
            blk8 = a8[:, 20 * ts : 20 * (ts + T)].reshape(P, 20, T)
            out[c, :, ts : ts + T, CH8] = (
                blk8[:, IDX8, :].astype(np.float32).transpose(1, 0, 2))
            ts += T
    return out.reshape(B, N, 16, 4)


def _run(x, W0, b0, W1, W2, W3, trace=False, iter_lens=None):
    iter_lens = list(iter_lens or ITER_LENS)
    consts = _host_constants(
        np.asarray(W0, np.float32), np.asarray(b0, np.float32),
        np.asarray(W1, np.float32), np.asarray(W2, np.float32),
        np.asarray(W3, np.float32),
    )
    nc = _get_program(consts, iter_lens)
    xin = _prep_inputs(x, iter_lens)
    in_maps = [{"xin": xin[c]} for c in range(M_CORES)]
    kwargs = {}
    if trace:
        kwargs = dict(trace=True, trace_cores=[0])
    res = run_bass_kernel_spmd(nc, in_maps, list(range(M_CORES)), **kwargs)
    out = _reconstruct(res.results, iter_lens)
    return out, res


def kernel(x, W0, b0, W1, W2, W3):
    out, _ = _run(x, W0, b0, W1, W2, W3)
    return out


def kernel_traced(x, W0, b0, W1, W2, W3, iter_lens=None):
    """Like kernel(), but captures an NTFF profile; returns (out, results)."""
    import sys
    import types

    if "antenv.axon_hooks" not in sys.modules:
        mod = types.ModuleType("antenv.axon_hooks")
        _h = [None]
        mod.set_axon_ntff_profile_hook = lambda h: _h.__setitem__(0, h)
        mod.get_axon_ntff_profile_hook = lambda: _h[0]
        sys.modules["antenv.axon_hooks"] = mod
        if "/root/.axon_site" not in sys.path:
            sys.path.insert(0, "/root/.axon_site")
        from trn_agent_boot.trn_boot import _ntff_profile_via_ctypes

        mod.set_axon_ntff_profile_hook(
            _ntff_profile_via_ctypes("/opt/axon/libaxon_pjrt.so")
        )
    import concourse.bass_utils as bu

    bu.upload_artifacts = lambda tmpdir: "local://" + tmpdir
    return _run(x, W0, b0, W1, W2, W3, trace=True, iter_lens=iter_lens)



# revision 3
# speedup vs baseline: 1.0575x; 1.0575x over previous
"""Trainium2 Bass kernel: DGCNN Zernike-monomial interwiner (nn_DGCNN_8839042695322).

Computes, per point p=(x,y,z):
  out[.., 16, 4] = concat_l( einsum(zernike_monoms(p)[l], Wl) ) for l=0..3
All weights fold host-side into per-channel scalar immediates (program cached
per weight set).

Memory-bound; correctness gate rel_err < 2e-2. v2 design (measured facts):
  - Single T=1024 iteration (halves per-op fixed cost vs 2x512: DVE op costs
    are N/speed + 143cyc; semaphore ops ~130ns each).
  - fp8 e4m3 for 45/64 output rows (l0 x4, l2 x20, l3 units != anchor x21);
    fp16 for l1 (12) + l3 anchor unit (7). Simulated rel err 9.3e-3.
  - DVE TS keeps 2x speed with fp8 output (measured 0.54ns/elem wide) --
    cheaper than ACT (0.87) for wide fp8 blocks; TT-fp8 drops to 1x (avoid).
  - GPSIMD/Pool is net-negative: concurrent Pool TT inflates DVE ops 3.6x
    (measured 690->2486ns) even on disjoint tiles. Not used.
  - l3 anchor-unit rows are built directly into the f16 output with w3[anchor]
    folded into the base constants; other units are TS/ACT copies of them.
  - b2 tile row order [xy, yz, xz | z-term, x2-y2] so ACT's l2 unit copies
    split into an early 3-row part (products only, ready before n2) and a
    late 2-row part -- fills ACT's early idle, feeds DMA sooner.

Sharding: pure data parallel over batch across 8 NeuronCores.
"""

import numpy as np

import concourse.bacc as bacc
import concourse.tile as tile
from concourse import mybir
from concourse.bass_utils import run_bass_kernel_spmd

# Problem geometry (hardcoded per spec: x [32, 32768, 3] f32, 8 cores).
B, N, M_CORES = 32, 32768, 8
PTS_PER_CORE = B * N // M_CORES  # 131072
P = 128                          # SBUF partitions
T = PTS_PER_CORE // P            # 1024 points per partition, single tile

# Real spherical-harmonic constants (match reference).
C0 = 0.28209479177387814
C1 = 0.4886025119029199
C2_XY = 1.0925484305920792
C2_0 = 0.31539156525252005
C2_2 = 0.5462742152960396
C3_3 = 0.5900435899266435
C3_2 = 2.890611442640554
C3_1 = 0.4570457994644658
C3_0 = 0.3731763325901154
C3_P2 = 1.445305721320277

# b2 tile row order: position -> mm (original l2 basis index)
B2_POS2MM = [0, 1, 3, 2, 4]
B2_MM2POS = [0, 1, 3, 2, 4]  # self-inverse

N16 = 19  # f16 rows: 12 l1 + 7 l3 anchor
N8 = 45   # f8 rows: 4 l0 + 20 l2 + 21 l3 non-anchor

_cache: dict = {}


def _host_constants(W0, b0, W1, W2, W3):
    f64 = np.float64
    A0 = (C0 * W0[0].astype(f64) + b0.astype(f64)).astype(np.float32)
    B0 = (C0 * W0[1].astype(f64)).astype(np.float32)
    AA1 = (C1 * W1[0].astype(f64)).astype(np.float32)
    BB1 = (C1 * W1[1].astype(f64)).astype(np.float32)
    w2u = W2[0].astype(f64).astype(np.float32)
    w3u = W3[0].astype(f64).astype(np.float32)
    anchor = int(np.argmax(np.abs(w3u)))
    return dict(A0=A0, B0=B0, AA1=AA1, BB1=BB1, w2u=w2u, w3u=w3u,
                anchor=anchor)


def _build_program(consts):
    f16 = mybir.dt.float16
    f8 = mybir.dt.float8e4
    F = mybir.ActivationFunctionType
    ALU = mybir.AluOpType
    A0, B0 = consts["A0"], consts["B0"]
    AA1, BB1 = consts["AA1"], consts["BB1"]
    w2u, w3u = consts["w2u"], consts["w3u"]
    anc = consts["anchor"]
    w3 = float(w3u[anc])
    others = [u for u in range(4) if u != anc]

    nc = bacc.Bacc(
        "TRN2", target_bir_lowering=False, debug=False, num_devices=M_CORES
    )
    xin = nc.dram_tensor("xin", [P, 3 * T], f16, kind="ExternalInput").ap()
    y16 = nc.dram_tensor("y16", [P, N16 * T], f16, kind="ExternalOutput").ap()
    y8 = nc.dram_tensor("y8", [P, N8 * T], f8, kind="ExternalOutput").ap()

    with tile.TileContext(nc) as tc:
        with (
            tc.tile_pool(name="xp", bufs=1) as xp,
            tc.tile_pool(name="wk", bufs=1) as wk,
            tc.tile_pool(name="op", bufs=1) as op_,
        ):
            xt = xp.tile([P, 3 * T], f16, name="xt")
            # Split input load: px,py first so cxy/squares start early.
            nc.sync.dma_start(out=xt[:, 0 : 2 * T], in_=xin[:, 0 : 2 * T])
            nc.sync.dma_start(out=xt[:, 2 * T : 3 * T], in_=xin[:, 2 * T :])
            px, py, pz = xt[:, 0:T], xt[:, T : 2 * T], xt[:, 2 * T : 3 * T]

            def pl(tag, k=1, dt=f16):
                return wk.tile([P, k * T], dt, name=tag)

            x2, y2, z2 = pl("x2"), pl("y2"), pl("z2")
            n2a, n2, x2my2, cn2_0 = pl("n2a"), pl("n2"), pl("x2my2"), pl("cn2_0")
            cxy = pl("cxy", 2)
            cpq = pl("cpq", 2)
            b2 = pl("b2", 5)
            sp = pl("sp", 4)
            a3, b3, d3 = pl("a3"), pl("b3"), pl("d3")
            cnA, u5nC = pl("cnA"), pl("u5nC")
            czA, czB, czC = pl("czA"), pl("czB"), pl("czC")
            o16 = op_.tile([P, N16 * T], f16, name="o16")
            o8 = op_.tile([P, N8 * T], f8, name="o8")

            def r16(r, k=1):
                return o16[:, r * T : (r + k) * T]

            def r8(r, k=1):
                return o8[:, r * T : (r + k) * T]

            def b2r(r, k=1):
                return b2[:, r * T : (r + k) * T]

            def odma16(r0, r1):
                nc.sync.dma_start(
                    out=y16[:, r0 * T : r1 * T], in_=r16(r0, r1 - r0))

            def odma8(r0, r1):
                nc.sync.dma_start(
                    out=y8[:, r0 * T : r1 * T], in_=r8(r0, r1 - r0))

            STT = nc.vector.scalar_tensor_tensor
            TT_MUL = nc.vector.tensor_mul
            TT_ADD = nc.vector.tensor_add
            TT_SUB = nc.vector.tensor_sub

            def TS(out, in_, s1, s2=None):
                if s2 is None:
                    nc.vector.tensor_scalar(
                        out=out, in0=in_, scalar1=float(s1), scalar2=None,
                        op0=ALU.mult)
                else:
                    nc.vector.tensor_scalar(
                        out=out, in0=in_, scalar1=float(s1), scalar2=float(s2),
                        op0=ALU.mult, op1=ALU.add)

            # === ACT stream: squares first (fills ACT while DVE ramps) ===
            nc.scalar.activation(x2, px, F.Square)
            nc.scalar.activation(y2, py, F.Square)
            nc.scalar.activation(z2, pz, F.Square)

            # === DVE: l2 product bases (need only xt) ===
            TS(cxy, xt[:, 0 : 2 * T], C2_XY)   # [C*px, C*py]
            TT_MUL(b2r(0), cxy[:, 0:T], py)    # C*px*py   (mm0)
            TT_MUL(b2r(1), cxy[:, T : 2 * T], pz)  # C*py*pz (mm1)
            TT_MUL(b2r(2), cxy[:, 0:T], pz)    # C*px*pz   (mm3)

            # === ACT: early 3-row l2 unit copies (fp8), DMA per piece ===
            for u in range(4):
                nc.scalar.activation(
                    r8(4 + 5 * u, 3), b2r(0, 3), F.Copy, scale=float(w2u[u]))
                odma8(4 + 5 * u, 7 + 5 * u)

            # === DVE: n2 chain + remaining b2 rows ===
            TT_SUB(x2my2, x2, y2)
            TT_ADD(n2a, x2, y2)
            TT_ADD(n2, n2a, z2)
            TS(cn2_0, n2, C2_0)
            STT(b2r(3), z2, 3.0 * C2_0, cn2_0, op0=ALU.mult, op1=ALU.subtract)
            TS(b2r(4), x2my2, C2_2)

            # === DVE: l0 rows (fp8, affine in n2) ===
            for u in range(4):
                nc.vector.tensor_scalar(
                    out=r8(u), in0=n2, scalar1=float(B0[u]),
                    scalar2=float(A0[u]), op0=ALU.mult, op1=ALU.add)
            odma8(0, 4)

            # === ACT: late 2-row l2 unit copies (fp8) ===
            for u in range(4):
                nc.scalar.activation(
                    r8(7 + 5 * u, 2), b2r(3, 2), F.Copy, scale=float(w2u[u]))
                odma8(7 + 5 * u, 9 + 5 * u)

            # === DVE: l3 anchor-unit bases, half 1 (rows 12..14) ===
            STT(a3, x2my2, 2.0, n2a, op0=ALU.mult, op1=ALU.add)   # 3x2-y2
            TS(cpq, xt[:, 0 : 2 * T], C3_3 * w3)  # [c*px, c*py]
            TT_MUL(r16(12), cpq[:, T : 2 * T], a3)          # m9
            TS(czA, pz, w3 * C3_2 / C2_XY)
            TT_MUL(r16(13), czA, b2r(0))                    # m10
            TS(cnA, n2, C3_1 * w3)
            STT(u5nC, z2, 5.0 * C3_1 * w3, cnA, op0=ALU.mult, op1=ALU.subtract)
            TT_MUL(r16(14), py, u5nC)                       # m11
            odma16(12, 15)

            # === DVE: sp + l1 (f16) ===
            for u in range(4):
                nc.vector.tensor_scalar(
                    out=sp[:, u * T : (u + 1) * T], in0=n2,
                    scalar1=float(BB1[u]), scalar2=float(AA1[u]),
                    op0=ALU.mult, op1=ALU.add)
            sp3 = sp.rearrange("p (a b) -> p a b", a=4)
            TT_MUL(r16(0, 4).rearrange("p (a b) -> p a b", a=4),
                   sp3, py.unsqueeze(1).broadcast_to([P, 4, T]))
            odma16(0, 4)
            TT_MUL(r16(4, 4).rearrange("p (a b) -> p a b", a=4),
                   sp3, pz.unsqueeze(1).broadcast_to([P, 4, T]))
            odma16(4, 8)
            TT_MUL(r16(8, 4).rearrange("p (a b) -> p a b", a=4),
                   sp3, px.unsqueeze(1).broadcast_to([P, 4, T]))
            odma16(8, 12)

            # === ACT: l3 unit copies, first parts (rows 12:15 ready) ===
            u1, u2, u3 = others
            s1, s2, s3 = (float(w3u[u1] / w3), float(w3u[u2] / w3),
                          float(w3u[u3] / w3))
            nc.scalar.activation(r8(24, 3), r16(12, 3), F.Copy, scale=s1)
            odma8(24, 27)

            # === DVE: l3 bases half 2 (rows 15..18) ===
            STT(d3, n2, 2.0 * C3_1 * w3, u5nC, op0=ALU.mult, op1=ALU.subtract)
            TS(czC, pz, -C3_0 / C3_1)
            TT_MUL(r16(15), czC, d3)                        # m12
            TT_MUL(r16(16), px, u5nC)                       # m13
            TS(czB, pz, w3 * C3_P2 / C2_2)
            TT_MUL(r16(17), czB, b2r(4))                    # m14
            STT(b3, x2my2, 2.0, n2a, op0=ALU.mult, op1=ALU.subtract)  # x2-3y2
            TT_MUL(r16(18), cpq[:, 0:T], b3)                # m15
            odma16(15, 19)

            # === ACT: l3 unit u3 first part (rows 12:15) ===
            nc.scalar.activation(r8(38, 3), r16(12, 3), F.Copy, scale=s3)
            odma8(38, 41)

            # === DVE: l3 unit u2 via TS-f8 (2x mode) ===
            nc.vector.tensor_scalar(
                out=r8(31, 7), in0=r16(12, 7), scalar1=s2, scalar2=None,
                op0=ALU.mult)

            # === ACT: l3 u1/u3 second parts (rows 15:19) ===
            nc.scalar.activation(r8(27, 4), r16(15, 4), F.Copy, scale=s1)
            odma8(27, 31)
            odma8(31, 38)
            nc.scalar.activation(r8(41, 4), r16(15, 4), F.Copy, scale=s3)
            odma8(41, 45)

    nc.compile()
    return nc


def _get_program(consts):
    key = tuple(
        consts[k].tobytes() for k in ("A0", "B0", "AA1", "BB1", "w2u", "w3u")
    ) + ("v2", consts["anchor"])
    if _cache.get(key) is None:
        _cache[key] = _build_program(consts)
    return _cache[key]


def _prep_inputs(x):
    """[B,N,3] f32 -> per-core [P, 3*T] fp16 with px|py|pz column blocks."""
    xs = np.asarray(x, dtype=np.float32).reshape(M_CORES, P, T, 3)
    xs = np.ascontiguousarray(xs.transpose(0, 1, 3, 2)).astype(np.float16)
    return xs.reshape(M_CORES, P, 3 * T)


def _make_index_maps(anchor):
    """Channel ch=(m*4+u) -> (which array, row)."""
    others = [u for u in range(4) if u != anchor]
    ch16, idx16, ch8, idx8 = [], [], [], []
    for ch in range(64):
        m, u = ch // 4, ch % 4
        if m == 0:
            ch8.append(ch); idx8.append(u)
        elif 1 <= m <= 3:
            ch16.append(ch); idx16.append(4 * (m - 1) + u)
        elif 4 <= m <= 8:
            mm = m - 4
            ch8.append(ch); idx8.append(4 + 5 * u + B2_MM2POS[mm])
        else:
            k = m - 9
            if u == anchor:
                ch16.append(ch); idx16.append(12 + k)
            else:
                j = others.index(u)
                ch8.append(ch); idx8.append(24 + 7 * j + k)
    return (np.array(ch16), np.array(idx16, dtype=np.int64),
            np.array(ch8), np.array(idx8, dtype=np.int64))


def _reconstruct(results, anchor):
    ch16, idx16, ch8, idx8 = _make_index_maps(anchor)
    out = np.empty((M_CORES, P, T, 64), dtype=np.float32)
    for c in range(M_CORES):
        a16 = results[c]["y16"].reshape(P, N16, T)
        a8 = results[c]["y8"].reshape(P, N8, T)
        out[c][:, :, ch16] = (
            a16[:, idx16, :].astype(np.float32).transpose(0, 2, 1))
        out[c][:, :, ch8] = (
            a8[:, idx8, :].astype(np.float32).transpose(0, 2, 1))
    return out.reshape(B, N, 16, 4)


def _run(x, W0, b0, W1, W2, W3, trace=False):
    consts = _host_constants(
        np.asarray(W0, np.float32), np.asarray(b0, np.float32),
        np.asarray(W1, np.float32), np.asarray(W2, np.float32),
        np.asarray(W3, np.float32),
    )
    nc = _get_program(consts)
    xin = _prep_inputs(x)
    in_maps = [{"xin": xin[c]} for c in range(M_CORES)]
    kwargs = {}
    if trace:
        kwargs = dict(trace=True, trace_cores=[0])
    res = run_bass_kernel_spmd(nc, in_maps, list(range(M_CORES)), **kwargs)
    out = _reconstruct(res.results, consts["anchor"])
    return out, res


def kernel(x, W0, b0, W1, W2, W3):
    out, _ = _run(x, W0, b0, W1, W2, W3)
    return out


def kernel_traced(x, W0, b0, W1, W2, W3):
    """Like kernel(), but captures an NTFF profile; returns (out, results)."""
    import sys
    import types

    if "antenv.axon_hooks" not in sys.modules:
        mod = types.ModuleType("antenv.axon_hooks")
        _h = [None]
        mod.set_axon_ntff_profile_hook = lambda h: _h.__setitem__(0, h)
        mod.get_axon_ntff_profile_hook = lambda: _h[0]
        sys.modules["antenv.axon_hooks"] = mod
        if "/root/.axon_site" not in sys.path:
            sys.path.insert(0, "/root/.axon_site")
        from trn_agent_boot.trn_boot import _ntff_profile_via_ctypes

        mod.set_axon_ntff_profile_hook(
            _ntff_profile_via_ctypes("/opt/axon/libaxon_pjrt.so")
        )
    import concourse.bass_utils as bu

    bu.upload_artifacts = lambda tmpdir: "local://" + tmpdir
    return _run(x, W0, b0, W1, W2, W3, trace=True)


# revision 4
# speedup vs baseline: 1.1331x; 1.0715x over previous
"""Trainium2 Bass kernel: DGCNN Zernike-monomial interwiner (nn_DGCNN_8839042695322).

Computes, per point p=(x,y,z):
  out[.., 16, 4] = concat_l( einsum(zernike_monoms(p)[l], Wl) ) for l=0..3
All weights fold host-side into per-channel scalar immediates (program cached
per weight set).

Memory-bound; correctness gate rel_err < 2e-2. v3 design notes (measured):
  - All compute ops run ~1.19x the isolated-probe cost once the 16 DMA
    engines stream concurrently (SBUF contention tax). ~11.4us of fixed
    preamble+input latency precedes the first compute op; ~4.5us of sem
    postamble trails the last DMA. Optimization target is therefore
    max(DVE, ACT, DMA) between those walls.
  - Single T=1024 iteration; DVE+ACT only (GPSIMD poisons concurrent DVE
    3.6x; Pool TS is 14ns/elem).
  - fp8 rows (40): l2 units != l2-anchor (15), l3 units != anchor (21),
    l0 (4). f16 rows (24): l1 (12), l3 anchor (7), l2 anchor (5).
    Simulated rel err ~9e-3 vs 2e-2 gate.
  - Anchor tricks: l3-anchor rows ARE the bl3 bases (w3[anchor] folded into
    base constants); l2-anchor rows ARE the b2 bases (w2[anchor2] folded).
    Other units are single scaled copies (ACT any-dtype 0.83ns/el; DVE
    TS-fp8 keeps 2x mode at 0.54ns/el).
  - Pair-merged TTs: [b2_3,b2_1] via pz broadcast, [m15,m9], [m10,m14],
    [m13,m11] via row ordering chosen so operands are adjacent/broadcast.

Sharding: pure data parallel over batch across 8 NeuronCores.
"""

import numpy as np

import concourse.bacc as bacc
import concourse.tile as tile
from concourse import mybir
from concourse.bass_utils import run_bass_kernel_spmd

B, N, M_CORES = 32, 32768, 8
PTS_PER_CORE = B * N // M_CORES  # 131072
P = 128
T = PTS_PER_CORE // P            # 1024

C0 = 0.28209479177387814
C1 = 0.4886025119029199
C2_XY = 1.0925484305920792
C2_0 = 0.31539156525252005
C2_2 = 0.5462742152960396
C3_3 = 0.5900435899266435
C3_2 = 2.890611442640554
C3_1 = 0.4570457994644658
C3_0 = 0.3731763325901154
C3_P2 = 1.445305721320277

# b2 tile row position -> mm (l2 basis index): [xz, yz, xy, x2-y2, z-term]
B2_MM2POS = [2, 1, 4, 0, 3]
# l3 base row position -> k (m-9): [m15, m9, m10, m14, m13, m11, m12]
L3_K2POS = [1, 2, 5, 6, 4, 3, 0]

# f16 rows: 0:12 l1, 12:19 l3 anchor bases, 19:24 l2 anchor (=b2 tile)
N16 = 24
# f8 rows: 0:15 l2 non-anchor units (3 blocks of 5), 15:36 l3 non-anchor
# units (3 blocks of 7), 36:40 l0
N8 = 40

_cache: dict = {}


def _host_constants(W0, b0, W1, W2, W3):
    f64 = np.float64
    A0 = (C0 * W0[0].astype(f64) + b0.astype(f64)).astype(np.float32)
    B0 = (C0 * W0[1].astype(f64)).astype(np.float32)
    AA1 = (C1 * W1[0].astype(f64)).astype(np.float32)
    BB1 = (C1 * W1[1].astype(f64)).astype(np.float32)
    w2u = W2[0].astype(f64).astype(np.float32)
    w3u = W3[0].astype(f64).astype(np.float32)
    anc3 = int(np.argmax(np.abs(w3u)))
    anc2 = int(np.argmax(np.abs(w2u)))
    return dict(A0=A0, B0=B0, AA1=AA1, BB1=BB1, w2u=w2u, w3u=w3u,
                anc3=anc3, anc2=anc2)


def _build_program(consts):
    f16 = mybir.dt.float16
    f8 = mybir.dt.float8e4
    F = mybir.ActivationFunctionType
    ALU = mybir.AluOpType
    A0, B0 = consts["A0"], consts["B0"]
    AA1, BB1 = consts["AA1"], consts["BB1"]
    w2u, w3u = consts["w2u"], consts["w3u"]
    anc3, anc2 = consts["anc3"], consts["anc2"]
    w3 = float(w3u[anc3])
    w2 = float(w2u[anc2])
    oth3 = [u for u in range(4) if u != anc3]
    oth2 = [u for u in range(4) if u != anc2]

    nc = bacc.Bacc(
        "TRN2", target_bir_lowering=False, debug=False, num_devices=M_CORES
    )
    xin = nc.dram_tensor("xin", [P, 3 * T], f16, kind="ExternalInput").ap()
    y16 = nc.dram_tensor("y16", [P, N16 * T], f16, kind="ExternalOutput").ap()
    y8 = nc.dram_tensor("y8", [P, N8 * T], f8, kind="ExternalOutput").ap()

    with tile.TileContext(nc) as tc:
        with (
            tc.tile_pool(name="xp", bufs=1) as xp,
            tc.tile_pool(name="wk", bufs=1) as wk,
            tc.tile_pool(name="op", bufs=1) as op_,
        ):
            xt = xp.tile([P, 3 * T], f16, name="xt")
            nc.sync.dma_start(out=xt[:, 0 : 2 * T], in_=xin[:, 0 : 2 * T])
            nc.sync.dma_start(out=xt[:, 2 * T : 3 * T], in_=xin[:, 2 * T :])
            px, py, pz = xt[:, 0:T], xt[:, T : 2 * T], xt[:, 2 * T : 3 * T]
            xt2 = xt[:, 0 : 2 * T].rearrange("p (a b) -> p a b", a=2)

            def pl(tag, k=1):
                return wk.tile([P, k * T], f16, name=tag)

            x2, y2, z2 = pl("x2"), pl("y2"), pl("z2")
            n2a, n2, x2my2, cn2_0 = pl("n2a"), pl("n2"), pl("x2my2"), pl("cn")
            cxy = pl("cxy", 2)
            cpq = pl("cpq", 2)
            cz2 = pl("cz2", 2)   # [czA, czB]
            sp = pl("sp", 4)
            ab3 = pl("ab3", 2)   # [b3, a3]
            cnA, u5nC, d3, czC = pl("cnA"), pl("u5nC"), pl("d3"), pl("czC")
            o16 = op_.tile([P, N16 * T], f16, name="o16")
            o8 = op_.tile([P, N8 * T], f8, name="o8")

            def r16(r, k=1):
                return o16[:, r * T : (r + k) * T]

            def r8(r, k=1):
                return o8[:, r * T : (r + k) * T]

            def odma16(r0, r1):
                nc.sync.dma_start(
                    out=y16[:, r0 * T : r1 * T], in_=r16(r0, r1 - r0))

            def odma8(r0, r1):
                nc.sync.dma_start(
                    out=y8[:, r0 * T : r1 * T], in_=r8(r0, r1 - r0))

            STT = nc.vector.scalar_tensor_tensor
            TT_MUL = nc.vector.tensor_mul
            TT_ADD = nc.vector.tensor_add
            TT_SUB = nc.vector.tensor_sub

            def TS(out, in_, s1, s2=None, dst=None):
                if s2 is None:
                    nc.vector.tensor_scalar(
                        out=out, in0=in_, scalar1=float(s1), scalar2=None,
                        op0=ALU.mult)
                else:
                    nc.vector.tensor_scalar(
                        out=out, in0=in_, scalar1=float(s1), scalar2=float(s2),
                        op0=ALU.mult, op1=ALU.add)

            def bc2(v):
                return v.unsqueeze(1).broadcast_to([P, 2, T])

            # === ACT: squares (overlap DVE's product chain) ===
            nc.scalar.activation(x2, px, F.Square)
            nc.scalar.activation(y2, py, F.Square)
            nc.scalar.activation(z2, pz, F.Square)

            # === DVE: l2-anchor product rows (y16 rows 19,20,21) ===
            TS(cxy, xt[:, 0 : 2 * T], C2_XY * w2)       # [c*px, c*py]
            cxy2 = cxy.rearrange("p (a b) -> p a b", a=2)
            TT_MUL(r16(19, 2).rearrange("p (a b) -> p a b", a=2),
                   cxy2, bc2(pz))                       # [b2_3, b2_1]
            TT_MUL(r16(21), cxy[:, 0:T], py)            # b2_0
            odma16(19, 22)

            # === ACT: l2 non-anchor early copies (rows 19:22 -> f8) ===
            for j, u in enumerate(oth2):
                nc.scalar.activation(
                    r8(5 * j, 3), r16(19, 3), F.Copy,
                    scale=float(w2u[u] / w2))
                odma8(5 * j, 5 * j + 3)

            # === DVE: n2 chain + b2 rows 22,23 ===
            TT_SUB(x2my2, x2, y2)
            TT_ADD(n2a, x2, y2)
            TT_ADD(n2, n2a, z2)
            TS(r16(22), x2my2, C2_2 * w2)               # b2_4
            TS(cn2_0, n2, C2_0 * w2)
            STT(r16(23), z2, 3.0 * C2_0 * w2, cn2_0,
                op0=ALU.mult, op1=ALU.subtract)         # b2_2
            odma16(22, 24)

            # === ACT: l2 non-anchor late copies (rows 22:24 -> f8) ===
            for j, u in enumerate(oth2):
                nc.scalar.activation(
                    r8(5 * j + 3, 2), r16(22, 2), F.Copy,
                    scale=float(w2u[u] / w2))
                odma8(5 * j + 3, 5 * j + 5)

            # === DVE: l3 anchor bases (y16 rows 12:19, w3 folded) ===
            STT(ab3[:, 0:T], x2my2, 2.0, n2a,
                op0=ALU.mult, op1=ALU.subtract)         # b3 = x2-3y2
            STT(ab3[:, T : 2 * T], x2my2, 2.0, n2a,
                op0=ALU.mult, op1=ALU.add)              # a3 = 3x2-y2
            TS(cpq, xt[:, 0 : 2 * T], C3_3 * w3)        # [c*px, c*py]
            TT_MUL(r16(12, 2).rearrange("p (a b) -> p a b", a=2),
                   cpq.rearrange("p (a b) -> p a b", a=2),
                   ab3.rearrange("p (a b) -> p a b", a=2))  # [m15, m9]
            TS(cz2[:, 0:T], pz, C3_2 / C2_XY / w2 * w3)     # czA (x b2_0)
            TS(cz2[:, T : 2 * T], pz, C3_P2 / C2_2 / w2 * w3)  # czB (x b2_4)
            # [m10, m14] = [czA*b2_0, czB*b2_4]; b2_0,b2_4 = y16 rows 21,22
            TT_MUL(r16(14, 2).rearrange("p (a b) -> p a b", a=2),
                   cz2.rearrange("p (a b) -> p a b", a=2),
                   r16(21, 2).rearrange("p (a b) -> p a b", a=2))
            TS(cnA, n2, C3_1 * w3)
            STT(u5nC, z2, 5.0 * C3_1 * w3, cnA,
                op0=ALU.mult, op1=ALU.subtract)         # c31*w3*(5z2-n2)
            TT_MUL(r16(16, 2).rearrange("p (a b) -> p a b", a=2),
                   xt2, bc2(u5nC))                      # [m13, m11]
            STT(d3, n2, 2.0 * C3_1 * w3, u5nC,
                op0=ALU.mult, op1=ALU.subtract)         # -c31*w3*(5z2-3n2)
            TS(czC, pz, -C3_0 / C3_1)
            TT_MUL(r16(18), czC, d3)                    # m12
            odma16(12, 19)

            # === ACT: l3 non-anchor unit copies (f8) ===
            s_oth = [float(w3u[u] / w3) for u in oth3]
            nc.scalar.activation(r8(15, 7), r16(12, 7), F.Copy, scale=s_oth[0])
            odma8(15, 22)

            # === DVE: sp + l1 (f16 rows 0:12) ===
            for u in range(4):
                nc.vector.tensor_scalar(
                    out=sp[:, u * T : (u + 1) * T], in0=n2,
                    scalar1=float(BB1[u]), scalar2=float(AA1[u]),
                    op0=ALU.mult, op1=ALU.add)
            sp3 = sp.rearrange("p (a b) -> p a b", a=4)
            TT_MUL(r16(0, 4).rearrange("p (a b) -> p a b", a=4),
                   sp3, py.unsqueeze(1).broadcast_to([P, 4, T]))
            odma16(0, 4)
            TT_MUL(r16(4, 4).rearrange("p (a b) -> p a b", a=4),
                   sp3, pz.unsqueeze(1).broadcast_to([P, 4, T]))
            odma16(4, 8)

            # === ACT: l3 third unit (f8) ===
            nc.scalar.activation(r8(29, 7), r16(12, 7), F.Copy, scale=s_oth[2])
            odma8(29, 36)

            TT_MUL(r16(8, 4).rearrange("p (a b) -> p a b", a=4),
                   sp3, px.unsqueeze(1).broadcast_to([P, 4, T]))
            odma16(8, 12)

            # === DVE: l3 second unit via TS-f8 (2x mode) ===
            nc.vector.tensor_scalar(
                out=r8(22, 7), in0=r16(12, 7), scalar1=s_oth[1], scalar2=None,
                op0=ALU.mult)
            odma8(22, 29)

            # === DVE: l0 (f8 rows 36:40, small tail chunk) ===
            for u in range(4):
                nc.vector.tensor_scalar(
                    out=r8(36 + u), in0=n2, scalar1=float(B0[u]),
                    scalar2=float(A0[u]), op0=ALU.mult, op1=ALU.add)
            odma8(36, 40)

    nc.compile()
    return nc


def _get_program(consts):
    key = tuple(
        consts[k].tobytes() for k in ("A0", "B0", "AA1", "BB1", "w2u", "w3u")
    ) + ("v3", consts["anc3"], consts["anc2"])
    if _cache.get(key) is None:
        _cache[key] = _build_program(consts)
    return _cache[key]


def _prep_inputs(x):
    xs = np.asarray(x, dtype=np.float32).reshape(M_CORES, P, T, 3)
    xs = np.ascontiguousarray(xs.transpose(0, 1, 3, 2)).astype(np.float16)
    return xs.reshape(M_CORES, P, 3 * T)


def _make_index_maps(anc3, anc2):
    oth3 = [u for u in range(4) if u != anc3]
    oth2 = [u for u in range(4) if u != anc2]
    ch16, idx16, ch8, idx8 = [], [], [], []
    for ch in range(64):
        m, u = ch // 4, ch % 4
        if m == 0:
            ch8.append(ch); idx8.append(36 + u)
        elif 1 <= m <= 3:
            ch16.append(ch); idx16.append(4 * (m - 1) + u)
        elif 4 <= m <= 8:
            mm = m - 4
            if u == anc2:
                ch16.append(ch); idx16.append(19 + B2_MM2POS[mm])
            else:
                j = oth2.index(u)
                ch8.append(ch); idx8.append(5 * j + B2_MM2POS[mm])
        else:
            k = m - 9
            if u == anc3:
                ch16.append(ch); idx16.append(12 + L3_K2POS[k])
            else:
                j = oth3.index(u)
                ch8.append(ch); idx8.append(15 + 7 * j + L3_K2POS[k])
    return (np.array(ch16), np.array(idx16, dtype=np.int64),
            np.array(ch8), np.array(idx8, dtype=np.int64))


def _reconstruct(results, anc3, anc2):
    ch16, idx16, ch8, idx8 = _make_index_maps(anc3, anc2)
    out = np.empty((M_CORES, P, T, 64), dtype=np.float32)
    for c in range(M_CORES):
        a16 = results[c]["y16"].reshape(P, N16, T)
        a8 = results[c]["y8"].reshape(P, N8, T)
        out[c][:, :, ch16] = (
            a16[:, idx16, :].astype(np.float32).transpose(0, 2, 1))
        out[c][:, :, ch8] = (
            a8[:, idx8, :].astype(np.float32).transpose(0, 2, 1))
    return out.reshape(B, N, 16, 4)


def _run(x, W0, b0, W1, W2, W3, trace=False):
    consts = _host_constants(
        np.asarray(W0, np.float32), np.asarray(b0, np.float32),
        np.asarray(W1, np.float32), np.asarray(W2, np.float32),
        np.asarray(W3, np.float32),
    )
    nc = _get_program(consts)
    xin = _prep_inputs(x)
    in_maps = [{"xin": xin[c]} for c in range(M_CORES)]
    kwargs = {}
    if trace:
        kwargs = dict(trace=True, trace_cores=[0])
    res = run_bass_kernel_spmd(nc, in_maps, list(range(M_CORES)), **kwargs)
    out = _reconstruct(res.results, consts["anc3"], consts["anc2"])
    return out, res


def kernel(x, W0, b0, W1, W2, W3):
    out, _ = _run(x, W0, b0, W1, W2, W3)
    return out


def kernel_traced(x, W0, b0, W1, W2, W3):
    import sys
    import types

    if "antenv.axon_hooks" not in sys.modules:
        mod = types.ModuleType("antenv.axon_hooks")
        _h = [None]
        mod.set_axon_ntff_profile_hook = lambda h: _h.__setitem__(0, h)
        mod.get_axon_ntff_profile_hook = lambda: _h[0]
        sys.modules["antenv.axon_hooks"] = mod
        if "/root/.axon_site" not in sys.path:
            sys.path.insert(0, "/root/.axon_site")
        from trn_agent_boot.trn_boot import _ntff_profile_via_ctypes

        mod.set_axon_ntff_profile_hook(
            _ntff_profile_via_ctypes("/opt/axon/libaxon_pjrt.so")
        )
    import concourse.bass_utils as bu

    bu.upload_artifacts = lambda tmpdir: "local://" + tmpdir
    return _run(x, W0, b0, W1, W2, W3, trace=True)


# revision 5
# speedup vs baseline: 1.2650x; 1.1163x over previous
"""Trainium2 Bass kernel: DGCNN Zernike-monomial interwiner (nn_DGCNN_8839042695322).

Computes, per point p=(x,y,z):
  out[.., 16, 4] = concat_l( einsum(zernike_monoms(p)[l], Wl) ) for l=0..3
All weights fold host-side into per-channel scalar immediates (program cached
per weight set).

Memory-bound; correctness gate rel_err < 2e-2. v3 design notes (measured):
  - All compute ops run ~1.19x the isolated-probe cost once the 16 DMA
    engines stream concurrently (SBUF contention tax). ~11.4us of fixed
    preamble+input latency precedes the first compute op; ~4.5us of sem
    postamble trails the last DMA. Optimization target is therefore
    max(DVE, ACT, DMA) between those walls.
  - Single T=1024 iteration; DVE+ACT only (GPSIMD poisons concurrent DVE
    3.6x; Pool TS is 14ns/elem).
  - fp8 rows (40): l2 units != l2-anchor (15), l3 units != anchor (21),
    l0 (4). f16 rows (24): l1 (12), l3 anchor (7), l2 anchor (5).
    Simulated rel err ~9e-3 vs 2e-2 gate.
  - Anchor tricks: l3-anchor rows ARE the bl3 bases (w3[anchor] folded into
    base constants); l2-anchor rows ARE the b2 bases (w2[anchor2] folded).
    Other units are single scaled copies (ACT any-dtype 0.83ns/el; DVE
    TS-fp8 keeps 2x mode at 0.54ns/el).
  - Pair-merged TTs: [b2_3,b2_1] via pz broadcast, [m15,m9], [m10,m14],
    [m13,m11] via row ordering chosen so operands are adjacent/broadcast.

Sharding: pure data parallel over batch across 8 NeuronCores.
"""

import numpy as np

import concourse.bacc as bacc
import concourse.tile as tile
from concourse import mybir
from concourse.bass_utils import run_bass_kernel_spmd

B, N, M_CORES = 32, 32768, 8
PTS_PER_CORE = B * N // M_CORES  # 131072
P = 128
T = PTS_PER_CORE // P            # 1024

C0 = 0.28209479177387814
C1 = 0.4886025119029199
C2_XY = 1.0925484305920792
C2_0 = 0.31539156525252005
C2_2 = 0.5462742152960396
C3_3 = 0.5900435899266435
C3_2 = 2.890611442640554
C3_1 = 0.4570457994644658
C3_0 = 0.3731763325901154
C3_P2 = 1.445305721320277

# b2 tile row position -> mm (l2 basis index): [xz, yz, xy, x2-y2, z-term]
B2_MM2POS = [2, 1, 4, 0, 3]
# l3 base row position -> k (m-9): [m15, m9, m10, m14, m13, m11, m12]
L3_K2POS = [1, 2, 5, 6, 4, 3, 0]

# f16 rows: 0:6 l1 u2/u3 (2 per m), 6:13 l3 anchor bases, 13:18 l2 anchor
N16 = 18
# f8 rows: 0:15 l2 non-anchor units, 15:36 l3 non-anchor units (3x7),
# 36:40 l0, 40:46 l1 u0/u1 (2 per m)
N8 = 46

_cache: dict = {}


def _host_constants(W0, b0, W1, W2, W3):
    f64 = np.float64
    A0 = (C0 * W0[0].astype(f64) + b0.astype(f64)).astype(np.float32)
    B0 = (C0 * W0[1].astype(f64)).astype(np.float32)
    AA1 = (C1 * W1[0].astype(f64)).astype(np.float32)
    BB1 = (C1 * W1[1].astype(f64)).astype(np.float32)
    w2u = W2[0].astype(f64).astype(np.float32)
    w3u = W3[0].astype(f64).astype(np.float32)
    anc3 = int(np.argmax(np.abs(w3u)))
    anc2 = int(np.argmax(np.abs(w2u)))
    return dict(A0=A0, B0=B0, AA1=AA1, BB1=BB1, w2u=w2u, w3u=w3u,
                anc3=anc3, anc2=anc2)


def _build_program(consts):
    f16 = mybir.dt.float16
    f8 = mybir.dt.float8e4
    F = mybir.ActivationFunctionType
    ALU = mybir.AluOpType
    A0, B0 = consts["A0"], consts["B0"]
    AA1, BB1 = consts["AA1"], consts["BB1"]
    w2u, w3u = consts["w2u"], consts["w3u"]
    anc3, anc2 = consts["anc3"], consts["anc2"]
    w3 = float(w3u[anc3])
    w2 = float(w2u[anc2])
    oth3 = [u for u in range(4) if u != anc3]
    oth2 = [u for u in range(4) if u != anc2]

    nc = bacc.Bacc(
        "TRN2", target_bir_lowering=False, debug=False, num_devices=M_CORES
    )
    xin = nc.dram_tensor("xin", [P, 3 * T], f16, kind="ExternalInput").ap()
    y16 = nc.dram_tensor("y16", [P, N16 * T], f16, kind="ExternalOutput").ap()
    y8 = nc.dram_tensor("y8", [P, N8 * T], f8, kind="ExternalOutput").ap()

    with tile.TileContext(nc) as tc:
        with (
            tc.tile_pool(name="xp", bufs=1) as xp,
            tc.tile_pool(name="wk", bufs=1) as wk,
            tc.tile_pool(name="op", bufs=1) as op_,
        ):
            xt = xp.tile([P, 3 * T], f16, name="xt")
            nc.sync.dma_start(out=xt[:, 0:T], in_=xin[:, 0:T])
            nc.sync.dma_start(out=xt[:, T : 2 * T], in_=xin[:, T : 2 * T])
            nc.sync.dma_start(out=xt[:, 2 * T : 3 * T], in_=xin[:, 2 * T :])
            px, py, pz = xt[:, 0:T], xt[:, T : 2 * T], xt[:, 2 * T : 3 * T]
            xt2 = xt[:, 0 : 2 * T].rearrange("p (a b) -> p a b", a=2)

            def pl(tag, k=1):
                return wk.tile([P, k * T], f16, name=tag)

            x2, y2, z2 = pl("x2"), pl("y2"), pl("z2")
            n2a, n2, x2my2, cn2_0 = pl("n2a"), pl("n2"), pl("x2my2"), pl("cn")
            cxy = pl("cxy", 2)
            cpq = pl("cpq", 2)
            cz2 = pl("cz2", 2)   # [czA, czB]
            sp = pl("sp", 4)
            ab3 = pl("ab3", 2)   # [b3, a3]
            cnA, u5nC, d3, czC = pl("cnA"), pl("u5nC"), pl("d3"), pl("czC")
            o16 = op_.tile([P, N16 * T], f16, name="o16")
            o8 = op_.tile([P, N8 * T], f8, name="o8")

            def r16(r, k=1):
                return o16[:, r * T : (r + k) * T]

            def r8(r, k=1):
                return o8[:, r * T : (r + k) * T]

            def odma16(r0, r1):
                nc.sync.dma_start(
                    out=y16[:, r0 * T : r1 * T], in_=r16(r0, r1 - r0))

            def odma8(r0, r1):
                nc.sync.dma_start(
                    out=y8[:, r0 * T : r1 * T], in_=r8(r0, r1 - r0))

            STT = nc.vector.scalar_tensor_tensor
            TT_MUL = nc.vector.tensor_mul
            TT_ADD = nc.vector.tensor_add
            TT_SUB = nc.vector.tensor_sub

            def TS(out, in_, s1, s2=None, dst=None):
                if s2 is None:
                    nc.vector.tensor_scalar(
                        out=out, in0=in_, scalar1=float(s1), scalar2=None,
                        op0=ALU.mult)
                else:
                    nc.vector.tensor_scalar(
                        out=out, in0=in_, scalar1=float(s1), scalar2=float(s2),
                        op0=ALU.mult, op1=ALU.add)

            def bc2(v):
                return v.unsqueeze(1).broadcast_to([P, 2, T])

            # === ACT: squares (overlap DVE's product chain) ===
            nc.scalar.activation(x2, px, F.Square)
            nc.scalar.activation(y2, py, F.Square)
            nc.scalar.activation(z2, pz, F.Square)

            # === DVE: l2-anchor product rows (y16 rows 19,20,21) ===
            TS(cxy, xt[:, 0 : 2 * T], C2_XY * w2)       # [c*px, c*py]
            cxy2 = cxy.rearrange("p (a b) -> p a b", a=2)
            TT_MUL(r16(13, 2).rearrange("p (a b) -> p a b", a=2),
                   cxy2, bc2(pz))                       # [b2_3, b2_1]
            TT_MUL(r16(15), cxy[:, 0:T], py)            # b2_0
            odma16(13, 16)

            # === ACT: l2 non-anchor early copies (rows 19:22 -> f8) ===
            for j, u in enumerate(oth2):
                nc.scalar.activation(
                    r8(5 * j, 3), r16(13, 3), F.Copy,
                    scale=float(w2u[u] / w2))
                odma8(5 * j, 5 * j + 3)

            # === DVE: n2 chain + b2 rows 22,23 ===
            TT_SUB(x2my2, x2, y2)
            TT_ADD(n2a, x2, y2)
            TT_ADD(n2, n2a, z2)
            TS(r16(16), x2my2, C2_2 * w2)               # b2_4
            TS(cn2_0, n2, C2_0 * w2)
            STT(r16(17), z2, 3.0 * C2_0 * w2, cn2_0,
                op0=ALU.mult, op1=ALU.subtract)         # b2_2
            odma16(16, 18)

            # === ACT: l2 non-anchor late copies (rows 22:24 -> f8) ===
            for j, u in enumerate(oth2):
                nc.scalar.activation(
                    r8(5 * j + 3, 2), r16(16, 2), F.Copy,
                    scale=float(w2u[u] / w2))
                odma8(5 * j + 3, 5 * j + 5)

            # === DVE: l3 anchor bases (y16 rows 12:19, w3 folded) ===
            STT(ab3[:, 0:T], x2my2, 2.0, n2a,
                op0=ALU.mult, op1=ALU.subtract)         # b3 = x2-3y2
            STT(ab3[:, T : 2 * T], x2my2, 2.0, n2a,
                op0=ALU.mult, op1=ALU.add)              # a3 = 3x2-y2
            TS(cpq, xt[:, 0 : 2 * T], C3_3 * w3)        # [c*px, c*py]
            TT_MUL(r16(6, 2).rearrange("p (a b) -> p a b", a=2),
                   cpq.rearrange("p (a b) -> p a b", a=2),
                   ab3.rearrange("p (a b) -> p a b", a=2))  # [m15, m9]
            TS(cz2[:, 0:T], pz, C3_2 / C2_XY / w2 * w3)     # czA (x b2_0)
            TS(cz2[:, T : 2 * T], pz, C3_P2 / C2_2 / w2 * w3)  # czB (x b2_4)
            # [m10, m14] = [czA*b2_0, czB*b2_4]; b2_0,b2_4 = y16 rows 21,22
            TT_MUL(r16(8, 2).rearrange("p (a b) -> p a b", a=2),
                   cz2.rearrange("p (a b) -> p a b", a=2),
                   r16(15, 2).rearrange("p (a b) -> p a b", a=2))
            TS(cnA, n2, C3_1 * w3)
            STT(u5nC, z2, 5.0 * C3_1 * w3, cnA,
                op0=ALU.mult, op1=ALU.subtract)         # c31*w3*(5z2-n2)
            TT_MUL(r16(10, 2).rearrange("p (a b) -> p a b", a=2),
                   xt2, bc2(u5nC))                      # [m13, m11]
            STT(d3, n2, 2.0 * C3_1 * w3, u5nC,
                op0=ALU.mult, op1=ALU.subtract)         # -c31*w3*(5z2-3n2)
            TS(czC, pz, -C3_0 / C3_1)
            TT_MUL(r16(12), czC, d3)                    # m12
            odma16(6, 13)

            # === ACT: l3 non-anchor unit copies (f8) ===
            s_oth = [float(w3u[u] / w3) for u in oth3]
            nc.scalar.activation(r8(15, 7), r16(6, 7), F.Copy, scale=s_oth[0])
            odma8(15, 22)

            # === DVE: sp + l1 (f16 rows 0:12) ===
            for u in range(4):
                nc.vector.tensor_scalar(
                    out=sp[:, u * T : (u + 1) * T], in0=n2,
                    scalar1=float(BB1[u]), scalar2=float(AA1[u]),
                    op0=ALU.mult, op1=ALU.add)
            spA = sp[:, 2 * T : 4 * T].rearrange("p (a b) -> p a b", a=2)
            spB = sp[:, 0 : 2 * T].rearrange("p (a b) -> p a b", a=2)
            TT_MUL(r16(0, 2).rearrange("p (a b) -> p a b", a=2),
                   spA, bc2(py))
            TT_MUL(r16(2, 2).rearrange("p (a b) -> p a b", a=2),
                   spA, bc2(pz))
            odma16(0, 4)

            # === ACT: l3 third unit (f8) ===
            nc.scalar.activation(r8(29, 7), r16(6, 7), F.Copy, scale=s_oth[2])
            odma8(29, 36)

            TT_MUL(r16(4, 2).rearrange("p (a b) -> p a b", a=2),
                   spA, bc2(px))
            odma16(4, 6)

            # === DVE: l3 second unit via TS-f8 (2x mode) ===
            nc.vector.tensor_scalar(
                out=r8(22, 7), in0=r16(6, 7), scalar1=s_oth[1], scalar2=None,
                op0=ALU.mult)
            odma8(22, 29)

            # === DVE: l1 u0/u1 (f8, TT 1x) ===
            TT_MUL(r8(40, 2).rearrange("p (a b) -> p a b", a=2), spB, bc2(py))
            TT_MUL(r8(42, 2).rearrange("p (a b) -> p a b", a=2), spB, bc2(pz))
            TT_MUL(r8(44, 2).rearrange("p (a b) -> p a b", a=2), spB, bc2(px))
            odma8(40, 46)

            # === DVE: l0 (f8 rows 36:40, small tail chunk) ===
            for u in range(4):
                nc.vector.tensor_scalar(
                    out=r8(36 + u), in0=n2, scalar1=float(B0[u]),
                    scalar2=float(A0[u]), op0=ALU.mult, op1=ALU.add)
            odma8(36, 40)

    nc.compile()
    return nc


def _get_program(consts):
    key = tuple(
        consts[k].tobytes() for k in ("A0", "B0", "AA1", "BB1", "w2u", "w3u")
    ) + ("v3", consts["anc3"], consts["anc2"])
    if _cache.get(key) is None:
        _cache[key] = _build_program(consts)
    return _cache[key]


def _prep_inputs(x):
    xs = np.asarray(x, dtype=np.float32).reshape(M_CORES, P, T, 3)
    xs = np.ascontiguousarray(xs.transpose(0, 1, 3, 2)).astype(np.float16)
    return xs.reshape(M_CORES, P, 3 * T)


def _make_index_maps(anc3, anc2):
    oth3 = [u for u in range(4) if u != anc3]
    oth2 = [u for u in range(4) if u != anc2]
    ch16, idx16, ch8, idx8 = [], [], [], []
    for ch in range(64):
        m, u = ch // 4, ch % 4
        if m == 0:
            ch8.append(ch); idx8.append(36 + u)
        elif 1 <= m <= 3:
            if u >= 2:
                ch16.append(ch); idx16.append(2 * (m - 1) + (u - 2))
            else:
                ch8.append(ch); idx8.append(40 + 2 * (m - 1) + u)
        elif 4 <= m <= 8:
            mm = m - 4
            if u == anc2:
                ch16.append(ch); idx16.append(13 + B2_MM2POS[mm])
            else:
                j = oth2.index(u)
                ch8.append(ch); idx8.append(5 * j + B2_MM2POS[mm])
        else:
            k = m - 9
            if u == anc3:
                ch16.append(ch); idx16.append(6 + L3_K2POS[k])
            else:
                j = oth3.index(u)
                ch8.append(ch); idx8.append(15 + 7 * j + L3_K2POS[k])
    return (np.array(ch16), np.array(idx16, dtype=np.int64),
            np.array(ch8), np.array(idx8, dtype=np.int64))


def _reconstruct(results, anc3, anc2):
    ch16, idx16, ch8, idx8 = _make_index_maps(anc3, anc2)
    out = np.empty((M_CORES, P, T, 64), dtype=np.float32)
    for c in range(M_CORES):
        a16 = results[c]["y16"].reshape(P, N16, T)
        a8 = results[c]["y8"].reshape(P, N8, T)
        out[c][:, :, ch16] = (
            a16[:, idx16, :].astype(np.float32).transpose(0, 2, 1))
        out[c][:, :, ch8] = (
            a8[:, idx8, :].astype(np.float32).transpose(0, 2, 1))
    return out.reshape(B, N, 16, 4)


def _run(x, W0, b0, W1, W2, W3, trace=False):
    consts = _host_constants(
        np.asarray(W0, np.float32), np.asarray(b0, np.float32),
        np.asarray(W1, np.float32), np.asarray(W2, np.float32),
        np.asarray(W3, np.float32),
    )
    nc = _get_program(consts)
    xin = _prep_inputs(x)
    in_maps = [{"xin": xin[c]} for c in range(M_CORES)]
    kwargs = {}
    if trace:
        kwargs = dict(trace=True, trace_cores=[0])
    res = run_bass_kernel_spmd(nc, in_maps, list(range(M_CORES)), **kwargs)
    out = _reconstruct(res.results, consts["anc3"], consts["anc2"])
    return out, res


def kernel(x, W0, b0, W1, W2, W3):
    out, _ = _run(x, W0, b0, W1, W2, W3)
    return out


def kernel_traced(x, W0, b0, W1, W2, W3):
    import sys
    import types

    if "antenv.axon_hooks" not in sys.modules:
        mod = types.ModuleType("antenv.axon_hooks")
        _h = [None]
        mod.set_axon_ntff_profile_hook = lambda h: _h.__setitem__(0, h)
        mod.get_axon_ntff_profile_hook = lambda: _h[0]
        sys.modules["antenv.axon_hooks"] = mod
        if "/root/.axon_site" not in sys.path:
            sys.path.insert(0, "/root/.axon_site")
        from trn_agent_boot.trn_boot import _ntff_profile_via_ctypes

        mod.set_axon_ntff_profile_hook(
            _ntff_profile_via_ctypes("/opt/axon/libaxon_pjrt.so")
        )
    import concourse.bass_utils as bu

    bu.upload_artifacts = lambda tmpdir: "local://" + tmpdir
    return _run(x, W0, b0, W1, W2, W3, trace=True)
